# revision 27
# baseline (speedup 1.0000x reference)
"""Multi-head attention forward on 8 Trainium2 NeuronCores (Bass/Tile).

Problem: B=2, S=2048, d_model=1024, 16 heads (depth 64), fp32.
  q/k/v = query @ W{q,k,v}; logits = q k^T / 8 + mask * -1e9;
  out = softmax(logits) v @ Wo.

Sharding (Megatron-style, hardcoded): core c handles batch b = c//4 and head
group hg = c%4 (4 heads = 256 of the 1024 head dims). Wq/Wk/Wv are
column-sharded, Wo row-sharded; each core emits a partial [S, 1024] output and
the host sums the 4 partials per batch (the "all-reduce").

Per-core kernel design (v2 — all-bf16 matmul path):
  * Every matmul operand lives in SBUF as bf16 (inputs are cast on the host),
    accumulation stays fp32 in PSUM. This doubles PE streaming throughput vs
    the fp32 path and enables fast weight load.
  * All attention math runs transposed: qT/kT are [depth, S] so QK^T lands
    as logitsT [k, q] tiles straight off the PE, and AV^T = V.T @ expT too.
  * ScalarE turns logits psum directly into exp weights (scale folded in);
    VectorE multiplies by (1-mask) in bf16 (2x mode).
  * The softmax denominator comes free from a ones-column appended to V
    (row 64 of the AV psum); VectorE takes its reciprocal straight off the
    psum row into a bf16 row-vector, which a rank-1 bf16 matmul broadcasts
    across the 64 head-depth partitions for the normalize multiply.
  * Output projection per 1024-wide q-chunk is folded into the main loop so
    it overlaps the next chunk's attention; output partials are written bf16.
  * A short identity-matmul spin warms the PE clock (HAM) while DMAs land.
"""

import sys

import numpy as np

sys.path.insert(0, "/opt/trn_rl_repo")

B = 2
S = 2048
D = 1024
HEADS = 16
DEPTH = 64
CORES = 8
HG = 4          # head groups (cores per batch)
HPC = 4         # heads per core
DH = HPC * DEPTH  # per-core head width = 256

_CACHE = {}


def _build_program():
    import concourse.bass as bass  # noqa: F401  (registers engines)
    import concourse.mybir as mybir
    import concourse.tile as tile
    from concourse import bacc
    from concourse.bass_interp import get_hw_module
    from concourse.masks import make_identity

    dt = mybir.dt
    f32, bf16 = dt.float32, dt.bfloat16
    MULT = mybir.AluOpType.mult
    EXP = mybir.ActivationFunctionType.Exp

    nc = bacc.Bacc(
        "TRN2",
        target_bir_lowering=False,
        debug=False,
        enable_asserts=True,
        num_devices=CORES,
    )

    xT = nc.dram_tensor("xT", [D, S], bf16, kind="ExternalInput").ap()
    imaskT = nc.dram_tensor("imaskT", [S, S], bf16, kind="ExternalInput").ap()
    wq = nc.dram_tensor("wq", [D, DH], bf16, kind="ExternalInput").ap()
    wk = nc.dram_tensor("wk", [D, DH], bf16, kind="ExternalInput").ap()
    wv = nc.dram_tensor("wv", [D, DH], bf16, kind="ExternalInput").ap()
    wo = nc.dram_tensor("wo", [DH, D], bf16, kind="ExternalInput").ap()
    vones = nc.dram_tensor("vones", [128, HPC, 1], bf16, kind="ExternalInput").ap()
    out = nc.dram_tensor("out", [S, D], bf16, kind="ExternalOutput").ap()

    with tile.TileContext(nc) as tc, \
         nc.allow_low_precision(reason="bf16 softmax weights; fp32 psum accumulate"):
        with tc.tile_pool(name="persist", bufs=1) as pp:
            # Persistent SBUF tiles.
            qT = [pp.tile([128, S], bf16, tag=f"qT{g}", name=f"qT{g}") for g in range(2)]
            kT = [pp.tile([128, S], bf16, tag=f"kT{g}", name=f"kT{g}") for g in range(2)]
            vt = [pp.tile([128, HPC, DEPTH + 1], bf16, tag=f"v{i}", name=f"v{i}") for i in range(16)]
            wot = pp.tile([128, 2, D], bf16, tag="wo", name="wo")
            ident = pp.tile([128, 128], f32, tag="ident", name="ident")
            ones_b = pp.tile([1, DEPTH], bf16, tag="ones_b", name="ones_b")
            vones_sb = pp.tile([128, HPC, 1], bf16, tag="vones", name="vones")

            make_identity(nc, ident[:])
            nc.gpsimd.memset(ones_b[:], 1.0)
            with tc.tile_pool(name="psW", bufs=2, space="PSUM") as psW:
                for w in range(48):
                    psw = psW.tile([128, 128], f32, tag="warm", name="warm")
                    nc.tensor.matmul(psw[:], ident[:], ident[:],
                                     start=True, stop=True)
            nc.sync.dma_start(wot[:], wo.rearrange("(g p) c -> p g c", p=128))
            # Mask tiles live for the whole kernel; their 8MB of DMAs are
            # issued during the projection phase so every tile has landed
            # before the attention loop consumes it.
            mt = pp.tile([128, 16, S], bf16, tag="mask", name="mask")
            imaskT_r = imaskT.rearrange("(t p) q -> p t q", p=128)

            # ---- Phase 1: projections (xT is query[b].T, host-cast bf16)
            with tc.tile_pool(name="xw", bufs=1) as xw, \
                 tc.tile_pool(name="psA", bufs=4, space="PSUM") as psA:
                xt = xw.tile([128, 8, S], bf16, tag="x", name="x")
                xT_r = xT.rearrange("(d p) s -> p d s", p=128)
                wts = {}
                for nm in ("wq", "wk", "wv"):
                    wts[nm] = xw.tile([128, 8, DH], bf16, tag=nm, name=nm)
                # Interleave weight and x-chunk DMAs so the first projection
                # units have their operands early (each dma_start costs ~650ns
                # of issue; transfers stripe across all 16 DMA engines).
                nc.sync.dma_start(wts["wq"][:], wq.rearrange("(d p) c -> p d c", p=128))
                nc.sync.dma_start(xt[:, 0:2, :], xT_r[:, 0:2, :])
                nc.sync.dma_start(wts["wk"][:], wk.rearrange("(d p) c -> p d c", p=128))
                nc.sync.dma_start(xt[:, 2:4, :], xT_r[:, 2:4, :])
                nc.sync.dma_start(wts["wv"][:], wv.rearrange("(d p) c -> p d c", p=128))
                nc.sync.dma_start(xt[:, 4:6, :], xT_r[:, 4:6, :])
                nc.sync.dma_start(xt[:, 6:8, :], xT_r[:, 6:8, :])
                nc.sync.dma_start(vones_sb[:], vones[:])
                # qT/kT: [dh, s] = Wq^T-slice . xT, accumulated over 8 D-chunks.
                ncopy = 0
                for wt, dst in ((wts["wq"], qT), (wts["wk"], kT)):
                    for g in range(2):
                        for sc in range(4):
                            ps = psA.tile([128, 512], f32, tag="proj", name="proj")
                            for d in range(8):
                                nc.tensor.matmul(
                                    ps[:],
                                    wt[:, d, g * 128:(g + 1) * 128],
                                    xt[:, d, sc * 512:(sc + 1) * 512],
                                    start=(d == 0), stop=(d == 7),
                                )
                            dst_ap = dst[g][:, sc * 512:(sc + 1) * 512]
                            if ncopy % 2 == 0:
                                nc.scalar.copy(dst_ap, ps[:])
                            else:
                                nc.vector.tensor_copy(dst_ap, ps[:])
                            ncopy += 1

                # v: natural [s, dh] layout, stored per 128-row tile as
                # [128, head, 65] with a ones column at index 64 (denominator).
                for st in range(16):
                    ps = psA.tile([128, DH], f32, tag="proj", name="proj")
                    for d in range(8):
                        nc.tensor.matmul(
                            ps[:],
                            xt[:, d, st * 128:(st + 1) * 128],
                            wts["wv"][:, d, :],
                            start=(d == 0), stop=(d == 7),
                        )
                    nc.gpsimd.tensor_copy(
                        vt[st][:, :, DEPTH:DEPTH + 1], vones_sb[:])
                    if st % 2 == 0:
                        nc.vector.tensor_copy(
                            vt[st][:, :, 0:DEPTH],
                            ps[:].rearrange("p (h e) -> p h e", h=HPC),
                        )
                    else:
                        nc.scalar.copy(
                            vt[st][:, :, 0:DEPTH],
                            ps[:].rearrange("p (h e) -> p h e", h=HPC),
                        )

            # ---- Phase 2: attention, fully transposed, all-bf16 ----
            attnT = [pp.tile([128, S], bf16, tag=f"attnT{g}", name=f"attnT{g}") for g in range(2)]
            with tc.tile_pool(name="attn", bufs=2) as ab, \
                 tc.tile_pool(name="exs", bufs=3) as exs, \
                 tc.tile_pool(name="psL", bufs=2, space="PSUM") as psL, \
                 tc.tile_pool(name="psO", bufs=3, space="PSUM") as psO, \
                 tc.tile_pool(name="psE", bufs=1, space="PSUM") as psE:
                for hb in range(2):
                    nc.sync.dma_start(mt[:, hb * 8:(hb + 1) * 8, :],
                                      imaskT_r[:, hb * 8:(hb + 1) * 8, :])
                # Two deferral queues keep the PE stream dense:
                #  * pending — per-head normalize (rank-1 + multiply), emitted a
                #    few kb-iterations into the NEXT head so the PE never waits
                #    on the reciprocal chain.
                #  * deferred — output-projection half-units (one psum tile +
                #    copy + DMA), drip-fed one per kb-iteration of the next
                #    q-chunk so the vector queue alternates em/ot work instead
                #    of stacking 32 copies ahead of the mask multiplies.
                pending = []
                deferred = []

                def outproj_half(st, nch):
                    def emit():
                        psf = psE.tile([128, 512], f32, tag="epi", name="po")
                        for g_ in range(2):
                            nc.tensor.matmul(
                                psf[:],
                                attnT[g_][:, st * 128:(st + 1) * 128],
                                wot[:, g_, nch * 512:(nch + 1) * 512],
                                start=(g_ == 0), stop=(g_ == 1),
                            )
                        ot = ab.tile([128, 512], bf16, tag="ot", name="ot", bufs=3)
                        if nch == 0:
                            nc.vector.tensor_copy(ot[:], psf[:])
                        else:
                            nc.scalar.copy(ot[:], psf[:])
                        nc.sync.dma_start(
                            out[st * 128:(st + 1) * 128,
                                nch * 512:(nch + 1) * 512], ot[:])
                    return emit

                def pe_filler():
                    # Dependency-free matmuls into an unread psum tile: keeps
                    # the PE activity monitor (HAM) from re-throttling the
                    # clock during scalar-paced stretches of the kb loop.
                    fl = psE.tile([128, 128], f32, tag="epi", name="fl")
                    for _ in range(5):
                        nc.tensor.matmul(fl[:], kT[0][:, 0:128], kT[0][:, 0:128],
                                         start=True, stop=True)

                def flush_pending():
                    while pending:
                        g_, po_, qcp_, rden_, last_h = pending.pop(0)
                        for half in range(2):
                            hs = slice(half * 512, (half + 1) * 512)
                            qh = slice(qcp_ * 1024 + half * 512,
                                       qcp_ * 1024 + half * 512 + 512)
                            psc = psE.tile([64, 512], f32, tag="epi", name="psc")
                            nc.tensor.matmul(
                                psc[:], ones_b[:], rden_[0:1, hs],
                                start=True, stop=True,
                            )
                            nc.vector.tensor_tensor(
                                attnT[g_][po_:po_ + 64, qh],
                                attnT[g_][po_:po_ + 64, qh], psc[:], MULT,
                            )
                        if last_h:
                            for st in range(qcp_ * 8, qcp_ * 8 + 8):
                                for nch in range(2):
                                    deferred.append(outproj_half(st, nch))

                # Per-head epilogue, staged across the next head's first kb
                # iterations so neither the scalar (exp) nor vector (em)
                # queue is blocked at the transition.  Slots:
                #   0: denominator-row copies (scalar, after next head's
                #      first exp) + attnT evictions (vector, after first em)
                #   1: reciprocal          2: bf16 cast -> rden
                #   3: handled by flush_pending (rank-1 + normalize TT)
                epi_stage = []

                def run_epi(slot):
                    for item in epi_stage:
                        for fn in item.pop(slot, []):
                            fn()

                def stage_epilogue(g, po, qcp, psoA, psoB, last_h):
                    qsl = slice(qcp * 1024, qcp * 1024 + 512)
                    qsr = slice(qcp * 1024 + 512, qcp * 1024 + 1024)
                    dden = ab.tile([1, 1024], f32, tag="dden", name="dden", bufs=2)
                    rdf = ab.tile([1, 1024], f32, tag="rdf", name="rdf", bufs=2)
                    rden = ab.tile([1, 1024], bf16, tag="rden", name="rden", bufs=2)

                    def s0():
                        nc.scalar.copy(dden[0:1, 0:512], psoA[64:65, :])
                        nc.scalar.copy(dden[0:1, 512:1024], psoB[64:65, :])
                        nc.vector.tensor_copy(attnT[g][po:po + 64, qsl], psoA[0:64, :])

                    def s1():
                        nc.vector.tensor_copy(attnT[g][po:po + 64, qsr], psoB[0:64, :])
                        nc.vector.reciprocal_approx_fast(rdf[:], dden[:])

                    def s2():
                        nc.vector.tensor_copy(rden[:], rdf[:])
                        pending.append((g, po, qcp, rden, last_h))

                    epi_stage.append({0: [s0], 1: [s1], 2: [s2]})

                def emit_qk(qcp, h, kb):
                    g = h // 2
                    po = (h % 2) * 64
                    psl = psL.tile([128, 1024], f32, tag="lg", name="lg")
                    for half in range(2):
                        hs = slice(half * 512, (half + 1) * 512)
                        qh = slice(qcp * 1024 + half * 512,
                                   qcp * 1024 + half * 512 + 512)
                        nc.tensor.matmul(
                            psl[:, hs],
                            kT[g][po:po + 64, kb * 128:(kb + 1) * 128],
                            qT[g][po:po + 64, qh],
                            start=True, stop=True,
                        )
                    return psl

                # Software-pipelined kb stream: QK for step i+1 is emitted
                # into the (in-order) PE queue BEFORE AV for step i, so the
                # exp for step i+1 never waits on step i's em chain.
                seq = [(qcp, h, kb)
                       for qcp in range(2) for h in range(HPC)
                       for kb in range(16)]
                psl_cur = emit_qk(*seq[0])
                psoA = psoB = None
                for idx, (qcp, h, kb) in enumerate(seq):
                    g, po = h // 2, (h % 2) * 64
                    qs = slice(qcp * 1024, (qcp + 1) * 1024)
                    if kb == 0:
                        psoA = psO.tile([65, 512], f32, tag="av", name="avA")
                        psoB = psO.tile([65, 512], f32, tag="av", name="avB")
                    psl_nxt = emit_qk(*seq[idx + 1]) if idx + 1 < len(seq) else None
                    if kb == 3:
                        flush_pending()
                    elif deferred and kb % 2 == 0 and kb >= 4:
                        deferred.pop(0)()
                    else:
                        pe_filler()
                    ex = exs.tile([128, 1024], bf16, tag="ex", name="ex", bufs=4)
                    nc.scalar.activation(ex[:], psl_cur[:], EXP, scale=0.125)
                    em = exs.tile([128, 1024], bf16, tag="em", name="em", bufs=8)
                    nc.vector.tensor_tensor(em[:], ex[:], mt[:, kb, qs], MULT)
                    if kb <= 2:
                        run_epi(kb)
                    for half in range(2):
                        hs = slice(half * 512, (half + 1) * 512)
                        nc.tensor.matmul(
                            psoA[:] if half == 0 else psoB[:],
                            vt[kb][:, h, :], em[:, hs],
                            start=(kb == 0), stop=(kb == 15),
                        )
                    psl_cur = psl_nxt
                    if kb == 15:
                        epi_stage.clear()
                        stage_epilogue(g, po, qcp, psoA, psoB, h == HPC - 1)

                # Tail: run the last head's staged epilogue and normalize,
                # then drain the remaining output-projection units.
                for slot in range(3):
                    run_epi(slot)
                flush_pending()
                while deferred:
                    deferred.pop(0)()

    nc.compile()
    nc.m = get_hw_module(nc.m)
    return nc


def _get_program():
    if "nc" not in _CACHE:
        _CACHE["nc"] = _build_program()
    return _CACHE["nc"]


def _make_in_maps(query, attention_mask, Wq, Wk, Wv, Wo):
    import ml_dtypes

    bf = ml_dtypes.bfloat16
    in_maps = []
    imaskT_b = []
    xT_b = []
    for b in range(B):
        imaskT_b.append(
            np.ascontiguousarray(1 - attention_mask[b, 0].T).astype(bf)
        )
        xT_b.append(np.ascontiguousarray(query[b].T).astype(bf))
    for c in range(CORES):
        b, hg = c // HG, c % HG
        cs = slice(hg * DH, (hg + 1) * DH)
        in_maps.append({
            "xT": xT_b[b],
            "imaskT": imaskT_b[b],
            "wq": np.ascontiguousarray(Wq[:, cs]).astype(bf),
            "wk": np.ascontiguousarray(Wk[:, cs]).astype(bf),
            "wv": np.ascontiguousarray(Wv[:, cs]).astype(bf),
            "wo": np.ascontiguousarray(Wo[cs, :]).astype(bf),
            "vones": np.ones((128, HPC, 1), dtype=bf),
        })
    return in_maps


def _run(inputs, trace=False):
    from concourse.bass_utils import run_bass_kernel_spmd

    nc = _get_program()
    in_maps = _make_in_maps(**inputs)
    res = run_bass_kernel_spmd(
        nc, in_maps, core_ids=list(range(CORES)), trace=trace,
    )
    outs = [res.results[c]["out"].astype(np.float64) for c in range(CORES)]
    full = np.empty((B, S, D), dtype=np.float32)
    for b in range(B):
        acc = outs[4 * b]
        for hg in range(1, HG):
            acc = acc + outs[4 * b + hg]
        full[b] = acc.astype(np.float32)
    return full, res


def kernel(query, attention_mask, Wq, Wk, Wv, Wo):
    full, _ = _run(dict(
        query=np.asarray(query), attention_mask=np.asarray(attention_mask),
        Wq=np.asarray(Wq), Wk=np.asarray(Wk), Wv=np.asarray(Wv),
        Wo=np.asarray(Wo),
    ))
    return full
